# revision 1
# baseline (speedup 1.0000x reference)
"""Trainium2 Bass kernel for nn_BertSelfAttention_31963146617403.

Binary-quantized BERT self-attention (BitBERT-style). Returns
(context, attention_scores, value_scores, query_scores, key_scores).

Sharding: data-parallel over batch — 16 batches / 8 cores = 2 per core.
Every core runs the same program on its own 2 batches; weights replicated.

Per-core pipeline (per batch):
  - PE-transpose x [512,768] -> xT [768,512] (fp32, via identity matmul)
  - fp32 projections qT/kT/vT = quantW @ xT + bias  (exact signs), stored
    as float32r (single-pass matmul dtype; rounding never flips a sign)
  - per head:
      * sign tensors P_q=[sign(q); ones], P_k=[sign(k); 8*mask] in bf16,
        so scores-psum = qb@kb^T + 8*mask and a single x0.125 copy-out
        yields attention_scores exactly
      * scoresT pass (operands swapped) -> 2*probs = clip(psum+1, 0, 2),
        exact in bf16 ({0,1,2}; psum is an even integer + 8*mask)
      * vb_nat = 0.5*sign(v) via matmul with 0.5*I (bf16)
      * context = (2*probs)^T-tiles @ vb_nat accumulated over k-chunks —
        bit-exact (products in {0,+-0.5,+-1}, fp32 psum)
      * query/key/value scores: float32r matmuls, x0.125 on copy-out
"""
import os
from contextlib import ExitStack

import numpy as np
import ml_dtypes

import concourse.bass as bass
import concourse.tile as tile
import concourse.mybir as mybir
from concourse import bacc
from concourse.bass_utils import run_bass_kernel_spmd

dt = mybir.dt
AF = mybir.ActivationFunctionType
ALU = mybir.AluOpType

NH, HS = 12, 64
H = NH * HS           # 768
S = 512
B = 16
NCORES = 8
BPC = B // NCORES     # batches per core
SCALE = 0.125         # 1/sqrt(64)
NC_CHUNKS = H // 128  # 6


def build_program():
    nc = bacc.Bacc("TRN2", target_bir_lowering=False, debug=False)

    hs_d = nc.dram_tensor("hs", [BPC, S, H], dt.float32, kind="ExternalInput").ap()
    wt_d = {
        t: nc.dram_tensor(f"w{t}t", [H, H], dt.float32, kind="ExternalInput").ap()
        for t in "qkv"
    }
    bias_d = {
        t: nc.dram_tensor(f"b{t}", [128, NC_CHUNKS], dt.float32, kind="ExternalInput").ap()
        for t in "qkv"
    }
    m8_d = nc.dram_tensor("mask8", [BPC, S], dt.float32, kind="ExternalInput").ap()
    id_d = nc.dram_tensor("ident", [128, 128], dt.float32, kind="ExternalInput").ap()
    hid_d = nc.dram_tensor("hident", [64, 64], dt.bfloat16, kind="ExternalInput").ap()

    ctx_d = nc.dram_tensor("ctx", [BPC, S, H], dt.float32, kind="ExternalOutput").ap()
    att_d = nc.dram_tensor("att", [BPC, NH, S, S], dt.float32, kind="ExternalOutput").ap()
    vsc_d = nc.dram_tensor("vsc", [BPC, NH, S, S], dt.float32, kind="ExternalOutput").ap()
    qsc_d = nc.dram_tensor("qsc", [BPC, NH, S, S], dt.float32, kind="ExternalOutput").ap()
    ksc_d = nc.dram_tensor("ksc", [BPC, NH, S, S], dt.float32, kind="ExternalOutput").ap()

    with tile.TileContext(nc) as tc, ExitStack() as ex:
        wp = ex.enter_context(tc.tile_pool(name="wp", bufs=1))
        xp = ex.enter_context(tc.tile_pool(name="xp", bufs=2))
        xtp = ex.enter_context(tc.tile_pool(name="xtp", bufs=6))
        qp = ex.enter_context(tc.tile_pool(name="qp", bufs=6))
        sp = ex.enter_context(tc.tile_pool(name="sp", bufs=3))
        pbp = ex.enter_context(tc.tile_pool(name="pbp", bufs=8))
        vnp = ex.enter_context(tc.tile_pool(name="vnp", bufs=8))
        op_ = ex.enter_context(tc.tile_pool(name="op", bufs=6))
        cp = ex.enter_context(tc.tile_pool(name="cp", bufs=1))
        ps_pj = ex.enter_context(tc.tile_pool(name="ps_pj", bufs=2, space="PSUM"))
        ps_big = ex.enter_context(tc.tile_pool(name="ps_big", bufs=4, space="PSUM"))
        ps_cx = ex.enter_context(tc.tile_pool(name="ps_cx", bufs=2, space="PSUM"))

        ident = wp.tile([128, 128], dt.float32, tag="ident", name="ident")
        nc.sync.dma_start(ident[:], id_d[:])
        hident = wp.tile([64, 64], dt.bfloat16, tag="hident", name="hident")
        nc.sync.dma_start(hident[:], hid_d[:])
        wtiles = {}
        for t in "qkv":
            for k in range(NC_CHUNKS):
                w = wp.tile([128, H], dt.float32, tag=f"w_{t}{k}", name=f"w_{t}{k}")
                nc.sync.dma_start(w[:], wt_d[t][k * 128:(k + 1) * 128, :])
                wtiles[t, k] = w
        btiles = {}
        for t in "qkv":
            bt = wp.tile([128, NC_CHUNKS], dt.float32, tag=f"b_{t}", name=f"b_{t}")
            nc.sync.dma_start(bt[:], bias_d[t][:])
            btiles[t] = bt

        for b in range(BPC):
            # ---- x transpose: xT[c,s] ----
            xts = [
                xtp.tile([128, S], dt.float32, tag="xt", name=f"xt{c}")
                for c in range(NC_CHUNKS)
            ]
            for sblk in range(4):
                xn = xp.tile([128, H], dt.float32, tag="xn", name="xn")
                nc.sync.dma_start(xn[:], hs_d[b, sblk * 128:(sblk + 1) * 128, :])
                for c in range(NC_CHUNKS):
                    tr = ps_pj.tile([128, 512], dt.float32, tag="pj", name="tr")
                    nc.tensor.transpose(
                        tr[:, 0:128], xn[:, c * 128:(c + 1) * 128], ident[:]
                    )
                    nc.any.tensor_copy(
                        xts[c][:, sblk * 128:(sblk + 1) * 128], tr[:, 0:128]
                    )

            # ---- projections (fp32 matmul, f32r output with bias) ----
            qkv = {}
            for t in "qkv":
                for j in range(NC_CHUNKS):
                    pj = ps_pj.tile([128, 512], dt.float32, tag="pj", name="pj")
                    for k in range(NC_CHUNKS):
                        nc.tensor.matmul(
                            pj[:],
                            wtiles[t, k][:, j * 128:(j + 1) * 128],
                            xts[k][:],
                            start=(k == 0),
                            stop=(k == NC_CHUNKS - 1),
                        )
                    qt = qp.tile([128, S], dt.float32r, tag=t, name=f"{t}{j}")
                    nc.vector.tensor_scalar(
                        qt[:], pj[:], btiles[t][:, j:j + 1], None, ALU.add
                    )
                    qkv[t, j] = qt

            # ---- mask row (8*mask in bf16) ----
            m8 = op_.tile([1, S], dt.float32, tag="m8", name="m8", bufs=2)
            nc.sync.dma_start(m8[:], m8_d[b:b + 1, :])
            m8b = op_.tile([1, S], dt.bfloat16, tag="m8b", name="m8b", bufs=2)
            nc.any.tensor_copy(m8b[:], m8[:])

            # ---- context accumulators for this batch ----
            ctx_sb = [
                cp.tile([128, H], dt.float32, tag=f"c{qs}", name=f"c{qs}")
                for qs in range(4)
            ]

            def emit_ctx(prev):
                hh, pbs, vns = prev
                for qs in range(4):
                    pcx = ps_cx.tile([128, 64], dt.float32, tag="cx", name="pcx")
                    for ks in range(4):
                        nc.tensor.matmul(
                            pcx[:],
                            pbs[ks][:, qs * 128:(qs + 1) * 128],
                            vns[ks][:],
                            start=(ks == 0),
                            stop=(ks == 3),
                        )
                    nc.any.tensor_copy(ctx_sb[qs][:, hh * 64:(hh + 1) * 64], pcx[:])

            prev = None
            for h in range(NH):
                j, po = h // 2, (h % 2) * 64
                qh = qkv["q", j][po:po + 64, :]
                kh = qkv["k", j][po:po + 64, :]
                vh = qkv["v", j][po:po + 64, :]

                pq = sp.tile([65, S], dt.bfloat16, tag="pq", name="pq")
                nc.scalar.activation(pq[0:64, :], qh, AF.Sign)
                nc.vector.memset(pq[64:65, :], 1.0)
                pk = sp.tile([65, S], dt.bfloat16, tag="pk", name="pk")
                nc.scalar.activation(pk[0:64, :], kh, AF.Sign)
                nc.vector.tensor_copy(pk[64:65, :], m8b[:])
                vbt = sp.tile([64, S], dt.bfloat16, tag="vb", name="vbt")
                nc.scalar.activation(vbt[:], vh, AF.Sign)

                # scores -> attention_scores
                for qs in range(4):
                    pscore = ps_big.tile([128, 512], dt.float32, tag="big", name="ps_s")
                    nc.tensor.matmul(
                        pscore[:], pq[:, qs * 128:(qs + 1) * 128], pk[:],
                        start=True, stop=True,
                    )
                    sc = op_.tile([128, S], dt.float32, tag="sc", name="sc")
                    nc.any.tensor_scalar(sc[:], pscore[:], SCALE, None, ALU.mult)
                    nc.sync.dma_start(att_d[b, h, qs * 128:(qs + 1) * 128, :], sc[:])

                # scoresT -> 2*probs (bf16, exact)
                pbs = []
                for ks in range(4):
                    pst = ps_big.tile([128, 512], dt.float32, tag="big", name="ps_t")
                    nc.tensor.matmul(
                        pst[:], pk[:, ks * 128:(ks + 1) * 128], pq[:],
                        start=True, stop=True,
                    )
                    t1 = op_.tile([128, S], dt.bfloat16, tag="t1", name="t1", bufs=4)
                    nc.any.tensor_scalar(t1[:], pst[:], 1.0, 0.0, ALU.add, ALU.max)
                    pb = pbp.tile([128, S], dt.bfloat16, tag="pb", name="pb")
                    nc.any.tensor_scalar(pb[:], t1[:], 2.0, None, ALU.min)
                    pbs.append(pb)

                # vb natural = 0.5*sign(v) [ks, d]
                vns = []
                for ks in range(4):
                    pv = ps_big.tile([128, 512], dt.float32, tag="big", name="ps_v")
                    nc.tensor.matmul(
                        pv[:, 0:64], vbt[:, ks * 128:(ks + 1) * 128], hident[:],
                        start=True, stop=True,
                    )
                    vn = vnp.tile([128, 64], dt.bfloat16, tag="vn", name="vn")
                    nc.any.tensor_copy(vn[:], pv[:, 0:64])
                    vns.append(vn)

                # query/key/value scores (f32r)
                for tname, od in (("q", qsc_d), ("k", ksc_d), ("v", vsc_d)):
                    th = qkv[tname, j][po:po + 64, :]
                    for qs in range(4):
                        p3 = ps_big.tile([128, 512], dt.float32, tag="big", name="ps_3")
                        nc.tensor.matmul(
                            p3[:], th[:, qs * 128:(qs + 1) * 128], th,
                            start=True, stop=True,
                        )
                        sc2 = op_.tile([128, S], dt.float32, tag="sc", name="sc2")
                        nc.any.tensor_scalar(sc2[:], p3[:], SCALE, None, ALU.mult)
                        nc.sync.dma_start(od[b, h, qs * 128:(qs + 1) * 128, :], sc2[:])

                if prev is not None:
                    emit_ctx(prev)
                prev = (h, pbs, vns)
            emit_ctx(prev)

            for qs in range(4):
                nc.sync.dma_start(ctx_d[b, qs * 128:(qs + 1) * 128, :], ctx_sb[qs][:])

    nc.compile()
    return nc


_NC_CACHE = None


def _get_nc():
    global _NC_CACHE
    if _NC_CACHE is None:
        _NC_CACHE = build_program()
    return _NC_CACHE


def prep_inputs(inputs):
    hs = np.ascontiguousarray(np.asarray(inputs["hidden_states"], dtype=np.float32))
    mask = np.asarray(inputs["attention_mask"], dtype=np.float32)
    m8 = np.ascontiguousarray((8.0 * mask[:, 0, 0, :]).astype(np.float32))
    shared = {
        "ident": np.eye(128, dtype=np.float32),
        "hident": (0.5 * np.eye(64)).astype(ml_dtypes.bfloat16),
    }
    for t in "qkv":
        W = np.asarray(inputs[f"W{t}"], dtype=np.float32)
        s = np.mean(np.abs(W), axis=1, keepdims=True).astype(np.float32)
        qw = (s * np.sign(W)).astype(np.float32)
        shared[f"w{t}t"] = np.ascontiguousarray(qw.T)
        bb = np.asarray(inputs[f"b{t}"], dtype=np.float32)
        shared[f"b{t}"] = np.ascontiguousarray(bb.reshape(NC_CHUNKS, 128).T)
    in_maps = []
    for c in range(NCORES):
        sl = slice(c * BPC, (c + 1) * BPC)
        in_maps.append({
            "hs": np.ascontiguousarray(hs[sl]),
            "mask8": np.ascontiguousarray(m8[sl]),
            **shared,
        })
    return in_maps


def assemble(results):
    ctx = np.empty((B, S, H), np.float32)
    att = np.empty((B, NH, S, S), np.float32)
    vsc = np.empty((B, NH, S, S), np.float32)
    qsc = np.empty((B, NH, S, S), np.float32)
    ksc = np.empty((B, NH, S, S), np.float32)
    for c in range(NCORES):
        sl = slice(c * BPC, (c + 1) * BPC)
        r = results[c]
        ctx[sl] = r["ctx"]
        att[sl] = r["att"]
        vsc[sl] = r["vsc"]
        qsc[sl] = r["qsc"]
        ksc[sl] = r["ksc"]
    return ctx, att, vsc, qsc, ksc


def run(inputs, trace=False, tmpdir=None):
    """Returns ((ctx, att, vsc, qsc, ksc), exec_time_ns)."""
    nc = _get_nc()
    in_maps = prep_inputs(inputs)
    out = run_bass_kernel_spmd(
        nc, in_maps, list(range(NCORES)), trace=trace, tmpdir=tmpdir
    )
    return assemble(out.results), out.exec_time_ns


def kernel(**inputs):
    return run(inputs)[0]
